# revision 15
# baseline (speedup 1.0000x reference)
"""Chamfer distance kernel for Trainium2 (8 NeuronCores).

Problem: pred/target [4, 8192, 3] f32 -> scalar
  mean_b( mean_m min_n ||p_bm - q_bn||^2 + mean_n min_m ||p_bm - q_bn||^2 )

Strategy (one "side" per core; 4 batches x 2 directions = 8 cores):
  Each core owns one (batch, direction) pair and computes, for each of its
  8192 "own" points, the min squared distance to all 8192 "other" points.

  Distances are produced on the TensorEngine as K=8 matmuls using the
  identity ||p-q||^2 = -2 p.q + ||p||^2 + ||q||^2:
      lhsT rows: [-2x, -2y, -2z, n_hi, n_lo, 1, 1, 0]   (own points)
      rhs  rows: [ x,   y,  z,  1,    1,  n_hi, n_lo, 0] (other points)
  Inputs are fp16; norms are split hi/lo into two fp16 values so the norm
  contribution keeps ~2^-22 precision; fp16 products are exact in the fp32
  PSUM accumulation. Because K=8 uses only 8 of the PE's 128 rows — and
  this part runs the PE cold at 1.2 GHz — four matmuls are packed into
  disjoint 32-row groups via tile_position, running concurrently (~4x).
  Host-side prep replicates lhsT/rhs at partition offsets 0/32/64/96.

  Each m-tile (128 own points) streams 4 "rounds" of 2048 distances into
  two rotating [128,2048] PSUM tiles (4 banks each). ScalarE stages 3
  rounds to fp16 SBUF; VectorE pair-mins (PSUM,staged) at 1x and
  (staged,staged) at fp16 2x, merges, and folds; per 8 m-tiles one batched
  fold+reduce produces the row minima. The 3-staged/1-direct split
  balances ScalarE vs VectorE (both ~equally busy, measured).
"""

import numpy as np

import concourse.bacc as bacc
import concourse.mybir as mybir
import concourse.tile as tile
from concourse import bass_utils

P = 128          # partitions / m-tile size
NPTS = 8192      # points per cloud
B = 4            # batch
K = 8            # matmul contraction (padded)
MT = NPTS // P   # 64 m-tiles
RND = 2048       # columns per round (one 4-bank PSUM tile, 4 packed MMs)
NRND = NPTS // RND  # 4 rounds per m-tile
MM_N = 512       # matmul free dim (one PSUM bank of fp32)
GRP = 8          # m-tiles whose tails are batched into one fold+reduce

F16 = mybir.dt.float16
F32 = mybir.dt.float32
MIN = mybir.AluOpType.min


def _emit_round(nc, ps, lt4, rt4, t, r):
    """4 row-group-packed K=8 matmuls filling one [128, 2048] PSUM tile."""
    for i in range(4):
        n0 = r * RND + i * MM_N
        nc.tensor.matmul(
            ps[:, i * MM_N:(i + 1) * MM_N],
            lt4[32 * i:32 * i + K, t * P:(t + 1) * P],
            rt4[32 * i:32 * i + K, n0:n0 + MM_N],
            start=True,
            stop=True,
            tile_position=(32 * i, 0),
        )


def _emit_mtile(nc, psum, stg, xpool, lt4, rt4, t, wbuf_slot):
    """One m-tile: 4 rounds of 2048 distances -> [128,512] fp16 partial
    mins in wbuf_slot (final fold+reduce happens per GRP m-tiles)."""
    # Round 0 staged; round 1 drained by DVE against st0 (fires early,
    # frees its PSUM banks mid-tile); rounds 2,3 staged for the 2x pair.
    ps0 = psum.tile([P, RND], F32, tag="ps")
    _emit_round(nc, ps0, lt4, rt4, t, 0)
    st0 = stg.tile([P, RND], F16, tag="st")
    nc.scalar.copy(st0[:], ps0[:])

    ps1 = psum.tile([P, RND], F32, tag="ps")
    _emit_round(nc, ps1, lt4, rt4, t, 1)
    x0 = xpool.tile([P, RND], F16, tag="x")
    nc.vector.tensor_tensor(x0[:], ps1[:], st0[:], op=MIN)

    staged = []
    for r in (2, 3):
        ps = psum.tile([P, RND], F32, tag="ps")
        _emit_round(nc, ps, lt4, rt4, t, r)
        st = stg.tile([P, RND], F16, tag="st")
        nc.scalar.copy(st[:], ps[:])
        staged.append(st)
    x1 = xpool.tile([P, RND], F16, tag="x")
    nc.vector.tensor_tensor(x1[:], staged[0][:], staged[1][:], op=MIN)
    # L2: merge; fold 2048 -> 1024 -> 512 into the group buffer slot.
    z = xpool.tile([P, RND], F16, tag="z")
    nc.vector.tensor_tensor(z[:], x0[:], x1[:], op=MIN)
    zz = xpool.tile([P, RND // 2], F16, tag="zz")
    nc.vector.tensor_tensor(zz[:], z[:, :RND // 2], z[:, RND // 2:], op=MIN)
    nc.vector.tensor_tensor(
        wbuf_slot, zz[:, :RND // 4], zz[:, RND // 4:], op=MIN
    )


def _build_nc():
    nc = bacc.Bacc(
        "TRN2", target_bir_lowering=False, debug=False, num_devices=8
    )
    lhsT_d = nc.dram_tensor("lhsT", [K, NPTS], F16, kind="ExternalInput")
    rhs_d = nc.dram_tensor("rhs", [K, NPTS], F16, kind="ExternalInput")
    mins_d = nc.dram_tensor("mins", [P, MT], F32, kind="ExternalOutput")

    with tile.TileContext(nc) as tc:
        with (
            tc.tile_pool(name="const", bufs=1) as const,
            tc.tile_pool(name="psum", bufs=2, space="PSUM") as psum,
            tc.tile_pool(name="stg", bufs=6) as stg,
            tc.tile_pool(name="xpool", bufs=4) as xpool,
            tc.tile_pool(name="wpool", bufs=2) as wpool,
        ):
            lt4 = const.tile([P, NPTS], F16)
            rt4 = const.tile([P, NPTS], F16)
            res = const.tile([P, MT], F32)
            # replicate the K=8 rows at partition offsets 0/32/64/96 for
            # row-group packing via 4 small DMAs per tensor
            for g in range(4):
                nc.sync.dma_start(lt4[32 * g:32 * g + K, :], lhsT_d.ap())
                nc.sync.dma_start(rt4[32 * g:32 * g + K, :], rhs_d.ap())

            W = MM_N  # wbuf slot width (512)
            for g in range(MT // GRP):
                wbuf = wpool.tile([P, GRP, W], F16, tag="w")
                for i in range(GRP):
                    t = g * GRP + i
                    _emit_mtile(
                        nc, psum, stg, xpool, lt4, rt4, t, wbuf[:, i, :]
                    )
                # batched tail: fold 512->256->128, reduce 128->1 per m-tile
                v = wpool.tile([P, GRP, W // 2], F16, tag="v")
                nc.vector.tensor_tensor(
                    v[:], wbuf[:, :, :W // 2], wbuf[:, :, W // 2:], op=MIN
                )
                u = wpool.tile([P, GRP, W // 4], F16, tag="u")
                nc.vector.tensor_tensor(
                    u[:], v[:, :, :W // 4], v[:, :, W // 4:], op=MIN
                )
                nc.vector.tensor_reduce(
                    res[:, g * GRP:(g + 1) * GRP], u[:],
                    axis=mybir.AxisListType.X, op=MIN,
                )

            nc.sync.dma_start(mins_d.ap(), res[:])

    nc.compile()
    return nc


_NC_CACHE = []


def _get_nc():
    if not _NC_CACHE:
        _NC_CACHE.append(_build_nc())
    return _NC_CACHE[0]


def _prep_side(own, other):
    """Build lhsT [8, N] (own) and rhs [8, N] (other) fp16 matmul inputs;
    the kernel replicates them to partition offsets 0/32/64/96 on-device."""
    o16 = own.astype(np.float16)
    t16 = other.astype(np.float16)
    o32 = o16.astype(np.float32)
    t32 = t16.astype(np.float32)
    on = (o32 * o32).sum(-1)       # fp32 norms of the fp16-rounded points
    tn = (t32 * t32).sum(-1)
    on_hi = on.astype(np.float16)
    on_lo = (on - on_hi.astype(np.float32)).astype(np.float16)
    tn_hi = tn.astype(np.float16)
    tn_lo = (tn - tn_hi.astype(np.float32)).astype(np.float16)

    n = own.shape[0]
    lhsT = np.zeros((K, n), np.float16)
    lhsT[0:3] = (-2.0 * o32).astype(np.float16).T
    lhsT[3] = on_hi
    lhsT[4] = on_lo
    lhsT[5] = 1.0
    lhsT[6] = 1.0
    rhs = np.zeros((K, n), np.float16)
    rhs[0:3] = t16.T
    rhs[3] = 1.0
    rhs[4] = 1.0
    rhs[5] = tn_hi
    rhs[6] = tn_lo
    return lhsT, rhs


def _in_maps_for(pred, target):
    pred = np.asarray(pred, dtype=np.float32)
    target = np.asarray(target, dtype=np.float32)
    in_maps = []
    for b in range(B):
        for d in range(2):
            own, other = (
                (pred[b], target[b]) if d == 0 else (target[b], pred[b])
            )
            lhsT, rhs = _prep_side(own, other)
            in_maps.append({"lhsT": lhsT, "rhs": rhs})
    return in_maps


def kernel(pred, target):
    in_maps = _in_maps_for(pred, target)
    nc = _get_nc()
    r = bass_utils.run_bass_kernel_spmd(nc, in_maps, core_ids=list(range(8)))

    total = 0.0
    for core_res in r.results:
        total += core_res["mins"].astype(np.float64).mean()
    return np.array(total / B, dtype=np.float32)


# revision 16
# speedup vs baseline: 1.1345x; 1.1345x over previous
"""Chamfer distance kernel for Trainium2 (8 NeuronCores).

Problem: pred/target [4, 8192, 3] f32 -> scalar
  mean_b( mean_m min_n ||p_bm - q_bn||^2 + mean_n min_m ||p_bm - q_bn||^2 )

Strategy (one "side" per core; 4 batches x 2 directions = 8 cores):
  Each core owns one (batch, direction) pair and computes, for each of its
  8192 "own" points, the min squared distance to all 8192 "other" points.

  Distances are produced on the TensorEngine as K=8 matmuls using the
  identity ||p-q||^2 = -2 p.q + ||p||^2 + ||q||^2:
      lhsT rows: [-2x, -2y, -2z, n_hi, n_lo, 1, 1, 0]   (own points)
      rhs  rows: [ x,   y,  z,  1,    1,  n_hi, n_lo, 0] (other points)
  Inputs are fp16; norms are split hi/lo into two fp16 values so the norm
  contribution keeps ~2^-22 precision; fp16 products are exact in the fp32
  PSUM accumulation. Because K=8 uses only 8 of the PE's 128 rows — and
  this part runs the PE cold at 1.2 GHz — four matmuls are packed into
  disjoint 32-row groups via tile_position, running concurrently (~4x).
  Host-side prep replicates lhsT/rhs at partition offsets 0/32/64/96.

  Each m-tile (128 own points) streams 4 "rounds" of 2048 distances into
  two rotating [128,2048] PSUM tiles (4 banks each). ScalarE stages 3
  rounds to fp16 SBUF; VectorE pair-mins (PSUM,staged) at 1x and
  (staged,staged) at fp16 2x, merges, and folds; per 8 m-tiles one batched
  fold+reduce produces the row minima. The 3-staged/1-direct split
  balances ScalarE vs VectorE (both ~equally busy, measured).
"""

import numpy as np

import concourse.bacc as bacc
import concourse.mybir as mybir
import concourse.tile as tile
from concourse import bass_utils

P = 128          # partitions / m-tile size
NPTS = 8192      # points per cloud
B = 4            # batch
K = 8            # matmul contraction (padded)
MT = NPTS // P   # 64 m-tiles
RND = 2048       # columns per round (one 4-bank PSUM tile, 4 packed MMs)
NRND = NPTS // RND  # 4 rounds per m-tile
MM_N = 512       # matmul free dim (one PSUM bank of fp32)
GRP = 8          # m-tiles whose tails are batched into one fold+reduce

F16 = mybir.dt.float16
F32 = mybir.dt.float32
MIN = mybir.AluOpType.min


def _emit_round(nc, ps, lt4, rt4, t, r):
    """4 row-group-packed K=8 matmuls filling one [128, 2048] PSUM tile."""
    for i in range(4):
        n0 = r * RND + i * MM_N
        nc.tensor.matmul(
            ps[:, i * MM_N:(i + 1) * MM_N],
            lt4[32 * i:32 * i + K, t * P:(t + 1) * P],
            rt4[32 * i:32 * i + K, n0:n0 + MM_N],
            start=True,
            stop=True,
            tile_position=(32 * i, 0),
        )


def _emit_mtile(nc, psum, stg, xpool, lt4, rt4, t, wbuf_slot):
    """One m-tile: 4 rounds of 2048 distances -> [128,512] fp16 partial
    mins in wbuf_slot (final fold+reduce happens per GRP m-tiles)."""
    # Rounds 0-2 staged to fp16 SBUF; round 3 drained by DVE against the
    # early-staged st0.
    staged = []
    for r in range(3):
        ps = psum.tile([P, RND], F32, tag="ps")
        _emit_round(nc, ps, lt4, rt4, t, r)
        st = stg.tile([P, RND], F16, tag="st")
        nc.scalar.copy(st[:], ps[:])
        staged.append(st)
    ps3 = psum.tile([P, RND], F32, tag="ps")
    _emit_round(nc, ps3, lt4, rt4, t, 3)

    x0 = xpool.tile([P, RND], F16, tag="x")
    nc.vector.tensor_tensor(x0[:], ps3[:], staged[0][:], op=MIN)
    x1 = xpool.tile([P, RND], F16, tag="x")
    nc.vector.tensor_tensor(x1[:], staged[1][:], staged[2][:], op=MIN)
    # L2: merge; fold 2048 -> 1024 -> 512 into the group buffer slot.
    z = xpool.tile([P, RND], F16, tag="z")
    nc.vector.tensor_tensor(z[:], x0[:], x1[:], op=MIN)
    zz = xpool.tile([P, RND // 2], F16, tag="zz")
    nc.vector.tensor_tensor(zz[:], z[:, :RND // 2], z[:, RND // 2:], op=MIN)
    nc.vector.tensor_tensor(
        wbuf_slot, zz[:, :RND // 4], zz[:, RND // 4:], op=MIN
    )


def _build_nc():
    nc = bacc.Bacc(
        "TRN2", target_bir_lowering=False, debug=False, num_devices=8
    )
    lhsT_d = nc.dram_tensor("lhsT", [K, NPTS], F16, kind="ExternalInput")
    rhs_d = nc.dram_tensor("rhs", [K, NPTS], F16, kind="ExternalInput")
    mins_d = nc.dram_tensor("mins", [P, MT], F32, kind="ExternalOutput")

    with tile.TileContext(nc) as tc:
        with (
            tc.tile_pool(name="const", bufs=1) as const,
            tc.tile_pool(name="psum", bufs=2, space="PSUM") as psum,
            tc.tile_pool(name="stg", bufs=6) as stg,
            tc.tile_pool(name="xpool", bufs=4) as xpool,
            tc.tile_pool(name="wpool", bufs=2) as wpool,
        ):
            lt4 = const.tile([P, NPTS], F16)
            rt4 = const.tile([P, NPTS], F16)
            res = const.tile([P, MT], F32)
            # replicate the K=8 rows at partition offsets 0/32/64/96 for
            # row-group packing via 4 small DMAs per tensor
            for g in range(4):
                nc.sync.dma_start(lt4[32 * g:32 * g + K, :], lhsT_d.ap())
                nc.sync.dma_start(rt4[32 * g:32 * g + K, :], rhs_d.ap())

            W = MM_N  # wbuf slot width (512)
            for g in range(MT // GRP):
                wbuf = wpool.tile([P, GRP, W], F16, tag="w")
                for i in range(GRP):
                    t = g * GRP + i
                    _emit_mtile(
                        nc, psum, stg, xpool, lt4, rt4, t, wbuf[:, i, :]
                    )
                # batched tail: fold 512->256->128, reduce 128->1 per m-tile
                v = wpool.tile([P, GRP, W // 2], F16, tag="v")
                nc.vector.tensor_tensor(
                    v[:], wbuf[:, :, :W // 2], wbuf[:, :, W // 2:], op=MIN
                )
                u = wpool.tile([P, GRP, W // 4], F16, tag="u")
                nc.vector.tensor_tensor(
                    u[:], v[:, :, :W // 4], v[:, :, W // 4:], op=MIN
                )
                nc.vector.tensor_reduce(
                    res[:, g * GRP:(g + 1) * GRP], u[:],
                    axis=mybir.AxisListType.X, op=MIN,
                )

            nc.sync.dma_start(mins_d.ap(), res[:])

    nc.compile()
    return nc


_NC_CACHE = []


def _get_nc():
    if not _NC_CACHE:
        _NC_CACHE.append(_build_nc())
    return _NC_CACHE[0]


def _prep_side(own, other):
    """Build lhsT [8, N] (own) and rhs [8, N] (other) fp16 matmul inputs;
    the kernel replicates them to partition offsets 0/32/64/96 on-device."""
    o16 = own.astype(np.float16)
    t16 = other.astype(np.float16)
    o32 = o16.astype(np.float32)
    t32 = t16.astype(np.float32)
    on = (o32 * o32).sum(-1)       # fp32 norms of the fp16-rounded points
    tn = (t32 * t32).sum(-1)
    on_hi = on.astype(np.float16)
    on_lo = (on - on_hi.astype(np.float32)).astype(np.float16)
    tn_hi = tn.astype(np.float16)
    tn_lo = (tn - tn_hi.astype(np.float32)).astype(np.float16)

    n = own.shape[0]
    lhsT = np.zeros((K, n), np.float16)
    lhsT[0:3] = (-2.0 * o32).astype(np.float16).T
    lhsT[3] = on_hi
    lhsT[4] = on_lo
    lhsT[5] = 1.0
    lhsT[6] = 1.0
    rhs = np.zeros((K, n), np.float16)
    rhs[0:3] = t16.T
    rhs[3] = 1.0
    rhs[4] = 1.0
    rhs[5] = tn_hi
    rhs[6] = tn_lo
    return lhsT, rhs


def _in_maps_for(pred, target):
    pred = np.asarray(pred, dtype=np.float32)
    target = np.asarray(target, dtype=np.float32)
    in_maps = []
    for b in range(B):
        for d in range(2):
            own, other = (
                (pred[b], target[b]) if d == 0 else (target[b], pred[b])
            )
            lhsT, rhs = _prep_side(own, other)
            in_maps.append({"lhsT": lhsT, "rhs": rhs})
    return in_maps


def kernel(pred, target):
    in_maps = _in_maps_for(pred, target)
    nc = _get_nc()
    r = bass_utils.run_bass_kernel_spmd(nc, in_maps, core_ids=list(range(8)))

    total = 0.0
    for core_res in r.results:
        total += core_res["mins"].astype(np.float64).mean()
    return np.array(total / B, dtype=np.float32)


# revision 17
# speedup vs baseline: 1.1424x; 1.0070x over previous
"""Chamfer distance kernel for Trainium2 (8 NeuronCores).

Problem: pred/target [4, 8192, 3] f32 -> scalar
  mean_b( mean_m min_n ||p_bm - q_bn||^2 + mean_n min_m ||p_bm - q_bn||^2 )

Strategy (one "side" per core; 4 batches x 2 directions = 8 cores):
  Each core owns one (batch, direction) pair and computes, for each of its
  8192 "own" points, the min squared distance to all 8192 "other" points.

  Distances are produced on the TensorEngine as K=8 matmuls using the
  identity ||p-q||^2 = -2 p.q + ||p||^2 + ||q||^2:
      lhsT rows: [-2x, -2y, -2z, n_hi, n_lo, 1, 1, 0]   (own points)
      rhs  rows: [ x,   y,  z,  1,    1,  n_hi, n_lo, 0] (other points)
  Inputs are fp16; norms are split hi/lo into two fp16 values so the norm
  contribution keeps ~2^-22 precision; fp16 products are exact in the fp32
  PSUM accumulation. Because K=8 uses only 8 of the PE's 128 rows — and
  this part runs the PE cold at 1.2 GHz — four matmuls are packed into
  disjoint 32-row groups via tile_position, running concurrently (~4x).
  Host-side prep replicates lhsT/rhs at partition offsets 0/32/64/96.

  Each m-tile (128 own points) streams 4 "rounds" of 2048 distances into
  two rotating [128,2048] PSUM tiles (4 banks each). ScalarE stages 3
  rounds to fp16 SBUF; VectorE pair-mins (PSUM,staged) at 1x and
  (staged,staged) at fp16 2x. The merge/fold ladder is batched across
  m-tile pairs, and the final fold+reduce across GRP=8 m-tiles, to
  amortize per-op overheads. The 3-staged/1-direct split balances
  ScalarE vs VectorE (both ~equally busy, measured).
"""

import numpy as np

import concourse.bacc as bacc
import concourse.mybir as mybir
import concourse.tile as tile
from concourse import bass_utils

P = 128          # partitions / m-tile size
NPTS = 8192      # points per cloud
B = 4            # batch
K = 8            # matmul contraction (padded)
MT = NPTS // P   # 64 m-tiles
RND = 2048       # columns per round (one 4-bank PSUM tile, 4 packed MMs)
MM_N = 512       # matmul free dim (one PSUM bank of fp32)
GRP = 8          # m-tiles whose tails are batched into one fold+reduce

F16 = mybir.dt.float16
F32 = mybir.dt.float32
MIN = mybir.AluOpType.min


def _emit_round(nc, ps, lt4, rt4, t, r):
    """4 row-group-packed K=8 matmuls filling one [128, 2048] PSUM tile."""
    for i in range(4):
        n0 = r * RND + i * MM_N
        nc.tensor.matmul(
            ps[:, i * MM_N:(i + 1) * MM_N],
            lt4[32 * i:32 * i + K, t * P:(t + 1) * P],
            rt4[32 * i:32 * i + K, n0:n0 + MM_N],
            start=True,
            stop=True,
            tile_position=(32 * i, 0),
        )


def _emit_mtile_l1(nc, psum, stg, lt4, rt4, t, x0_slot, x1_slot):
    """One m-tile's matmuls + L1 pair-mins into the pair buffers.

    Rounds 0-2 are staged to fp16 SBUF by ScalarE; round 3 is drained by
    VectorE against the early-staged st0 (1x); st1/st2 pair at fp16 2x.
    """
    staged = []
    for r in range(3):
        ps = psum.tile([P, RND], F32, tag="ps")
        _emit_round(nc, ps, lt4, rt4, t, r)
        st = stg.tile([P, RND], F16, tag="st")
        nc.scalar.copy(st[:], ps[:])
        staged.append(st)
    ps3 = psum.tile([P, RND], F32, tag="ps")
    _emit_round(nc, ps3, lt4, rt4, t, 3)

    nc.vector.tensor_tensor(x0_slot, ps3[:], staged[0][:], op=MIN)
    nc.vector.tensor_tensor(x1_slot, staged[1][:], staged[2][:], op=MIN)


def _build_nc():
    nc = bacc.Bacc(
        "TRN2", target_bir_lowering=False, debug=False, num_devices=8
    )
    lhsT_d = nc.dram_tensor("lhsT", [P, NPTS], F16, kind="ExternalInput")
    rhs_d = nc.dram_tensor("rhs", [P, NPTS], F16, kind="ExternalInput")
    mins_d = nc.dram_tensor("mins", [P, MT], F32, kind="ExternalOutput")

    with tile.TileContext(nc) as tc:
        with (
            tc.tile_pool(name="const", bufs=1) as const,
            tc.tile_pool(name="psum", bufs=2, space="PSUM") as psum,
            tc.tile_pool(name="stg", bufs=5) as stg,
            tc.tile_pool(name="xpool", bufs=2) as xpool,
            tc.tile_pool(name="wpool", bufs=2) as wpool,
        ):
            lt4 = const.tile([P, NPTS], F16)
            rt4 = const.tile([P, NPTS], F16)
            res = const.tile([P, MT], F32)
            nc.sync.dma_start(lt4[:], lhsT_d.ap())
            nc.sync.dma_start(rt4[:], rhs_d.ap())

            W = MM_N  # wbuf slot width (512)
            for g in range(MT // GRP):
                wbuf = wpool.tile([P, GRP, W], F16, tag="w")
                for j in range(GRP // 2):
                    # process an m-tile pair; batch its merge ladder
                    x0b = xpool.tile([P, 2, RND], F16, tag="x0")
                    x1b = xpool.tile([P, 2, RND], F16, tag="x1")
                    for h in range(2):
                        t = g * GRP + 2 * j + h
                        _emit_mtile_l1(
                            nc, psum, stg, lt4, rt4, t,
                            x0b[:, h, :], x1b[:, h, :],
                        )
                    z2 = xpool.tile([P, 2, RND], F16, tag="z2")
                    nc.vector.tensor_tensor(z2[:], x0b[:], x1b[:], op=MIN)
                    zz2 = xpool.tile([P, 2, RND // 2], F16, tag="zz2")
                    nc.vector.tensor_tensor(
                        zz2[:], z2[:, :, :RND // 2], z2[:, :, RND // 2:],
                        op=MIN,
                    )
                    nc.vector.tensor_tensor(
                        wbuf[:, 2 * j:2 * j + 2, :],
                        zz2[:, :, :RND // 4], zz2[:, :, RND // 4:], op=MIN,
                    )
                # batched tail: fold 512->256->128, reduce 128->1 per m-tile
                v = wpool.tile([P, GRP, W // 2], F16, tag="v")
                nc.vector.tensor_tensor(
                    v[:], wbuf[:, :, :W // 2], wbuf[:, :, W // 2:], op=MIN
                )
                u = wpool.tile([P, GRP, W // 4], F16, tag="u")
                nc.vector.tensor_tensor(
                    u[:], v[:, :, :W // 4], v[:, :, W // 4:], op=MIN
                )
                nc.vector.tensor_reduce(
                    res[:, g * GRP:(g + 1) * GRP], u[:],
                    axis=mybir.AxisListType.X, op=MIN,
                )

            nc.sync.dma_start(mins_d.ap(), res[:])

    nc.compile()
    return nc


_NC_CACHE = []


def _get_nc():
    if not _NC_CACHE:
        _NC_CACHE.append(_build_nc())
    return _NC_CACHE[0]


def _prep_side(own, other):
    """Build lhsT [128, N] and rhs [128, N] fp16 with the K=8 row content
    replicated at partition offsets 0/32/64/96 for row-group packing."""
    o16 = own.astype(np.float16)
    t16 = other.astype(np.float16)
    o32 = o16.astype(np.float32)
    t32 = t16.astype(np.float32)
    on = (o32 * o32).sum(-1)       # fp32 norms of the fp16-rounded points
    tn = (t32 * t32).sum(-1)
    on_hi = on.astype(np.float16)
    on_lo = (on - on_hi.astype(np.float32)).astype(np.float16)
    tn_hi = tn.astype(np.float16)
    tn_lo = (tn - tn_hi.astype(np.float32)).astype(np.float16)

    n = own.shape[0]
    lhsT = np.zeros((K, n), np.float16)
    lhsT[0:3] = (-2.0 * o32).astype(np.float16).T
    lhsT[3] = on_hi
    lhsT[4] = on_lo
    lhsT[5] = 1.0
    lhsT[6] = 1.0
    rhs = np.zeros((K, n), np.float16)
    rhs[0:3] = t16.T
    rhs[3] = 1.0
    rhs[4] = 1.0
    rhs[5] = tn_hi
    rhs[6] = tn_lo

    lhsT4 = np.zeros((P, n), np.float16)
    rhs4 = np.zeros((P, n), np.float16)
    for g in range(4):
        lhsT4[32 * g:32 * g + K] = lhsT
        rhs4[32 * g:32 * g + K] = rhs
    return lhsT4, rhs4


def _in_maps_for(pred, target):
    pred = np.asarray(pred, dtype=np.float32)
    target = np.asarray(target, dtype=np.float32)
    in_maps = []
    for b in range(B):
        for d in range(2):
            own, other = (
                (pred[b], target[b]) if d == 0 else (target[b], pred[b])
            )
            lhsT4, rhs4 = _prep_side(own, other)
            in_maps.append({"lhsT": lhsT4, "rhs": rhs4})
    return in_maps


def kernel(pred, target):
    in_maps = _in_maps_for(pred, target)
    nc = _get_nc()
    r = bass_utils.run_bass_kernel_spmd(nc, in_maps, core_ids=list(range(8)))

    total = 0.0
    for core_res in r.results:
        total += core_res["mins"].astype(np.float64).mean()
    return np.array(total / B, dtype=np.float32)
